# revision 27
# baseline (speedup 1.0000x reference)
"""Trainium2 Bass kernel for nn_MixtureOfRanksLayer (moe_routing).

Two-launch expert-parallel design (device does ALL the math; the host only
moves data between launches — slicing, gathering rows by the device-computed
routing, and summing per-expert partial outputs, i.e. the unshard step):

LAUNCH 1 — data-parallel over tokens (512/core on 8 cores):
  T1  = x @ u1           [512, E*R=512]   rank projections for all experts
  lg  = x @ gate_w.T+b   [512, 8]         gate logits (same lhsT tiles)
  w   = top2-renormalized weights (masked-max + sigmoid(l1-l2), exact,
        identical math to the softmax-top2-renorm reference)
  outputs: T1 (bf16), w (f32, nonzero exactly at the top-2 experts)

HOST between launches: from w>0 derive each expert's token list; gather the
64 T1 columns of expert e at its tokens into a [cap, 64] block (cap =
per-expert capacity rounded to 384-slot chunks, computed from the actual
counts so it never overflows); lay out chunk c on SBUF partition half c%2
so the rank-64 matmuls of consecutive chunks run CONCURRENTLY on the two
row/column halves of the PE array (measured: tiled pairs stream at full
aggregate rate, one 384-row pair costs one stream).

LAUNCH 2 — expert-parallel (core e owns expert e, weights are NOT
replicated: 3.9MB/core instead of 21MB, the DMA roofline win):
  h   = relu(v1.T @ T1g + b1)   per hc chunk, evac ACT/DVE alternating
  T2  = u2.T @ h                 accumulated over hc (col-tiled halves)
  y   = (T2.T @ v2 + b2) * w     w applied as per-partition scale at evac
  output: y [cap, 2048] bf16

HOST: out[idx_e] += y_e[:count]  (the expert-parallel unshard/combine).

Measured HW facts this design is built on (microbench, 8xNC-v3):
  bf16 matmul ~0.345ns/row; fp32r is 2.8x SLOWER (so everything is bf16);
  row/col-tiled K=64/M=64 pairs at different tile positions stream
  concurrently; per-core DMA ~180B/ns; ACT/DVE evac ~1 col/cycle (PSUM
  source blocks DVE 2x modes) which makes h-evacuation the main non-PE
  cost: minimized by capacity (~1152 slots vs 4096 dense tokens/expert).
"""

from contextlib import ExitStack, nullcontext

import ml_dtypes
import numpy as np

import concourse.bacc as bacc
import concourse.mybir as mybir
import concourse.tile as tile

dt = mybir.dt
AF = mybir.ActivationFunctionType
ALU = mybir.AluOpType
AX = mybir.AxisListType

E, D, H, R = 8, 2048, 8192, 64
N_TOK = 4096
NCORES = 8
NT = N_TOK // NCORES     # launch-1 tokens per core
DC = D // 128            # contraction chunks over d_model
ER = E * R               # stacked expert-rank axis
HC = H // 128            # hidden chunks
SC = 384                 # launch-2 slot chunk (psum: 384 f32 = 1.5KB/bank)
TC = NT // 128           # launch-1 token chunks

bf16 = ml_dtypes.bfloat16


# --------------------------------------------------------------------------
# Launch 1: T1 + routing weights, data-parallel
# --------------------------------------------------------------------------
def build_l1(rep=0):
    f32 = dt.float32
    bf = dt.bfloat16
    nc = bacc.Bacc("TRN2", debug=False)

    xt_d = nc.dram_tensor("xt", [128, DC * NT], bf, kind="ExternalInput").ap()
    xl_d = nc.dram_tensor("xtl", [128, DC * NT], bf, kind="ExternalInput").ap()
    u1_d = nc.dram_tensor("u1c", [128, DC * ER], bf, kind="ExternalInput").ap()
    gw_d = nc.dram_tensor("gwt", [128, DC * 2 * E], bf, kind="ExternalInput").ap()
    gb_d = nc.dram_tensor("gbb", [128, E], f32, kind="ExternalInput").ap()
    t1_d = nc.dram_tensor("t1", [NT, ER], bf, kind="ExternalOutput").ap()
    w_d = nc.dram_tensor("w", [NT, E], f32, kind="ExternalOutput").ap()

    with ExitStack() as ctx:
        tc = ctx.enter_context(tile.TileContext(nc))
        const = ctx.enter_context(tc.tile_pool(name="const", bufs=1))
        sm = ctx.enter_context(tc.tile_pool(name="sm", bufs=2))
        outp = ctx.enter_context(tc.tile_pool(name="outp", bufs=2))
        ps_t1 = ctx.enter_context(tc.tile_pool(name="ps_t1", bufs=2, space="PSUM"))
        ps_lg = ctx.enter_context(tc.tile_pool(name="ps_lg", bufs=2, space="PSUM"))

        loop = tc.For_i(0, rep) if rep else nullcontext()
        with loop:
            xt = const.tile([128, DC, NT], bf, tag="xt")
            nc.sync.dma_start(xt, xt_d)
            xtl = const.tile([128, DC, NT], bf, tag="xtl")
            nc.sync.dma_start(xtl, xl_d)
            u1 = const.tile([128, DC, ER], bf, tag="u1")
            nc.sync.dma_start(u1, u1_d)
            gw = const.tile([128, DC, 2 * E], bf, tag="gw")
            nc.sync.dma_start(gw, gw_d)
            gbb = const.tile([128, E], f32, tag="gbb")
            nc.sync.dma_start(gbb, gb_d)

            for t in range(TC):
                pt = ps_t1.tile([128, ER], f32, tag="pt")
                pl = ps_lg.tile([128, 2 * E], f32, tag="pl")
                pl3 = ps_lg.tile([128, E], f32, tag="pl3")
                for c in range(DC):
                    lhs = xt[:, c, t * 128:(t + 1) * 128]
                    nc.tensor.matmul(pt, lhsT=lhs, rhs=u1[:, c, :],
                                     start=(c == 0), stop=(c == DC - 1),
                                     skip_group_check=True)
                    # logits in split-bf16: xhi@[gwhi|gwlo] and xlo@gwhi
                    nc.tensor.matmul(pl, lhsT=lhs, rhs=gw[:, c, :],
                                     start=(c == 0), stop=(c == DC - 1),
                                     skip_group_check=True)
                    nc.tensor.matmul(pl3,
                                     lhsT=xtl[:, c, t * 128:(t + 1) * 128],
                                     rhs=gw[:, c, 0:E],
                                     start=(c == 0), stop=(c == DC - 1),
                                     skip_group_check=True)
                t1o = outp.tile([128, ER], bf, tag="t1o")
                nc.vector.tensor_copy(t1o, pt)
                nc.sync.dma_start(t1_d[t * 128:(t + 1) * 128, :], t1o)

                # routing: exact top-2 renormalized softmax weights
                lgs = sm.tile([128, 2 * E], f32, tag="lgs")
                nc.vector.tensor_copy(lgs, pl)
                lg1 = sm.tile([128, E], f32, tag="lg1")
                nc.vector.tensor_tensor(lg1, pl3, gbb, op=ALU.add)
                lg0 = sm.tile([128, E], f32, tag="lg0")
                nc.vector.tensor_tensor(lg0, lgs[:, 0:E], lgs[:, E:2 * E],
                                        op=ALU.add)
                lg = sm.tile([128, E], f32, tag="lg")
                nc.vector.tensor_add(lg, lg0, lg1)
                l1 = sm.tile([128, 1], f32, tag="l1")
                nc.vector.reduce_max(out=l1, in_=lg, axis=AX.X)
                m1t = sm.tile([128, E], f32, tag="m1t")
                nc.vector.tensor_scalar(m1t, lg, l1, None, op0=ALU.is_equal)
                lm = sm.tile([128, E], f32, tag="lm")
                nc.vector.tensor_scalar(lm, m1t, -1e30, None, op0=ALU.mult)
                nc.vector.tensor_add(lm, lm, lg)
                l2 = sm.tile([128, 1], f32, tag="l2")
                nc.vector.reduce_max(out=l2, in_=lm, axis=AX.X)
                m2t = sm.tile([128, E], f32, tag="m2t")
                nc.vector.tensor_scalar(m2t, lm, l2, None, op0=ALU.is_equal)
                dif = sm.tile([128, 1], f32, tag="dif")
                nc.vector.tensor_sub(dif, l1, l2)
                s1v = sm.tile([128, 1], f32, tag="s1v")
                nc.scalar.activation(s1v, dif, AF.Sigmoid)
                s0v = sm.tile([128, 1], f32, tag="s0v")
                nc.scalar.activation(s0v, dif, AF.Sigmoid, scale=-1.0)
                wa = sm.tile([128, E], f32, tag="wa")
                nc.vector.tensor_scalar(wa, m1t, s1v, None, op0=ALU.mult)
                wb_ = sm.tile([128, E], f32, tag="wb_")
                nc.vector.tensor_scalar(wb_, m2t, s0v, None, op0=ALU.mult)
                wt = outp.tile([128, E], f32, tag="wt")
                nc.vector.tensor_add(wt, wa, wb_)
                nc.sync.dma_start(w_d[t * 128:(t + 1) * 128, :], wt)

    nc.compile()
    return nc


# --------------------------------------------------------------------------
# Launch 2: per-expert FFN over gathered slots, expert-parallel
# --------------------------------------------------------------------------
def build_l2(nch, rep=0, with_b2=True):
    f32 = dt.float32
    bf = dt.bfloat16
    npair = (nch + 1) // 2
    cap = nch * SC
    nsub = nch * (SC // 128)
    DD = D // 512
    nc = bacc.Bacc("TRN2", debug=False)

    t1g_d = nc.dram_tensor("t1g", [128, npair * SC], bf, kind="ExternalInput").ap()
    v1_d = nc.dram_tensor("v1d", [128, H], bf, kind="ExternalInput").ap()
    u2_d = nc.dram_tensor("u2l", [128, HC * R], bf, kind="ExternalInput").ap()
    v2_d = nc.dram_tensor("v2d", [128, D], bf, kind="ExternalInput").ap()
    b1_d = nc.dram_tensor("b1l", [128, HC], f32, kind="ExternalInput").ap()
    b2_d = nc.dram_tensor("b2r", [1, D], bf, kind="ExternalInput").ap()
    wg_d = nc.dram_tensor("wgc", [128, nsub], f32, kind="ExternalInput").ap()
    on_d = nc.dram_tensor("onesk", [1, 128], bf, kind="ExternalInput").ap()
    y_d = nc.dram_tensor("y", [cap, D], bf, kind="ExternalOutput").ap()

    # chunk -> (pair tile index, partition half)
    halves = [(ch // 2, (ch % 2) * 64) for ch in range(nch)]

    with ExitStack() as ctx:
        tc = ctx.enter_context(tile.TileContext(nc))
        const = ctx.enter_context(tc.tile_pool(name="const", bufs=1))
        t2p = ctx.enter_context(tc.tile_pool(name="t2p", bufs=1))
        hsb = ctx.enter_context(tc.tile_pool(name="hsb", bufs=6))
        ysb = ctx.enter_context(tc.tile_pool(name="ysb", bufs=4))

        loop = tc.For_i(0, rep) if rep else nullcontext()
        with loop:
            t1g = const.tile([128, npair, SC], bf, tag="t1g")
            nc.sync.dma_start(t1g, t1g_d)
            v1d = const.tile([128, H], bf, tag="v1d")
            nc.sync.dma_start(v1d, v1_d)
            u2l = const.tile([128, HC, R], bf, tag="u2l")
            nc.sync.dma_start(u2l, u2_d)
            u2l2 = const.tile([128, HC, R], bf, tag="u2l2")
            nc.sync.dma_start(u2l2, u2_d)
            v2d = const.tile([128, D], bf, tag="v2d")
            nc.sync.dma_start(v2d, v2_d)
            b1l = const.tile([128, HC], f32, tag="b1l")
            nc.sync.dma_start(b1l, b1_d)
            b2r = const.tile([1, D], bf, tag="b2r")
            nc.sync.dma_start(b2r, b2_d)
            wgc = const.tile([128, nsub], f32, tag="wgc")
            nc.sync.dma_start(wgc, wg_d)
            onesk = const.tile([1, 128], bf, tag="onesk")
            nc.sync.dma_start(onesk, on_d)

            t2sb = t2p.tile([128, npair, SC], bf, tag="t2sb")

            # ---- phase A: h = relu(v1.T @ t1g + b1); T2 += u2.T @ h ----
            with ExitStack() as sA:
                ps_ha = sA.enter_context(
                    tc.tile_pool(name="ps_ha", bufs=3, space="PSUM"))
                ps_hb = sA.enter_context(
                    tc.tile_pool(name="ps_hb", bufs=3, space="PSUM"))
                ps_t2 = sA.enter_context(
                    tc.tile_pool(name="ps_t2", bufs=1, space="PSUM"))

                # software pipeline: m2+evac for hc runs DEPTH chunks ahead
                # of m3(hc), so the in-order PE never blocks on an evac it
                # just scheduled (evac ~525ns+2 sems vs ~250ns of PE work/hc)
                DEPTH = 3
                for pj in range(npair):
                    has_b = 2 * pj + 1 < nch
                    # separate PSUM banks per column half (same-bank col-tiled
                    # accumulation serializes the PE; separate banks stream)
                    pt2a = ps_t2.tile([128, SC], f32, tag="pt2a", name="pt2a")
                    pt2b = None
                    if has_b:
                        pt2b = ps_t2.tile([128, SC], f32, tag="pt2b", name="pt2b")
                    hq = {}
                    for step in range(HC + DEPTH):
                        if step < HC:
                            hc = step
                            hs = slice(hc * 128, (hc + 1) * 128)
                            pha = ps_ha.tile([128, SC], f32, tag="ha", name="ha")
                            nc.tensor.matmul(pha, lhsT=v1d[0:64, hs],
                                             rhs=t1g[0:64, pj, :],
                                             start=True, stop=True)
                            if has_b:
                                phb = ps_hb.tile([128, SC], f32, tag="hb",
                                                 name="hb")
                                nc.tensor.matmul(phb, lhsT=v1d[64:128, hs],
                                                 rhs=t1g[64:128, pj, :],
                                                 start=True, stop=True)
                            ha = hsb.tile([128, SC], bf, tag="ha", name="sha")
                            bias = b1l[:, hc:hc + 1]
                            if hc % 2 == 0:
                                nc.scalar.activation(ha, pha, AF.Relu, bias=bias)
                            else:
                                nc.vector.tensor_scalar(ha, pha, bias, 0.0,
                                                        op0=ALU.add, op1=ALU.max)
                            hb = None
                            if has_b:
                                hb = hsb.tile([128, SC], bf, tag="hb", name="shb")
                                if hc % 2 == 0:
                                    nc.vector.tensor_scalar(hb, phb, bias, 0.0,
                                                            op0=ALU.add,
                                                            op1=ALU.max)
                                else:
                                    nc.scalar.activation(hb, phb, AF.Relu,
                                                         bias=bias)
                            hq[hc] = (ha, hb)
                        mc = step - DEPTH
                        if mc >= 0:
                            ha, hb = hq.pop(mc)
                            nc.tensor.matmul(pt2a[0:64, :], lhsT=u2l[:, mc, :],
                                             rhs=ha,
                                             start=(mc == 0), stop=(mc == HC - 1),
                                             skip_group_check=True)
                            if hb is not None:
                                nc.tensor.matmul(pt2b[64:128, :],
                                                 lhsT=u2l2[:, mc, :], rhs=hb,
                                                 start=(mc == 0),
                                                 stop=(mc == HC - 1),
                                                 skip_group_check=True)
                    nc.vector.tensor_copy(t2sb[0:64, pj, :], pt2a[0:64, :])
                    if has_b:
                        nc.vector.tensor_copy(t2sb[64:128, pj, :],
                                              pt2b[64:128, :])

            # ---- phase B: y = (T2.T @ v2 + b2) * w ----
            # b2 matmuls only when b2 != 0 (K=1 LDWEIGHTS thrash is pricey);
            # one wide [128, D] output tile per slot-sub so the SP queue
            # dispatches 1 DMA/sub instead of 4 (565ns dispatch each)
            sB = ExitStack()
            ps_y = sB.enter_context(
                tc.tile_pool(name="ps_y", bufs=4, space="PSUM"))
            groups = [tuple(c for c in (2 * i, 2 * i + 1) if c < nch)
                      for i in range((nch + 1) // 2)]
            for grp in groups:
                for s3 in range(SC // 128):
                    yos = {}
                    for ch in grp:
                        sub = ch * (SC // 128) + s3
                        yos[ch] = ysb.tile([128, D], bf, tag="yo",
                                           name=f"yo{sub}")
                    for dd in range(DD):
                        pys = []
                        for ch in grp:
                            pj, half = halves[ch]
                            py = ps_y.tile([128, 512], f32, tag="py", name="py")
                            nc.tensor.matmul(
                                py,
                                lhsT=t2sb[half:half + 64, pj,
                                          s3 * 128:(s3 + 1) * 128],
                                rhs=v2d[half:half + 64, dd * 512:(dd + 1) * 512],
                                start=True, stop=not with_b2,
                                skip_group_check=True)
                            pys.append(py)
                        if with_b2:
                            for ch, py in zip(grp, pys):
                                nc.tensor.matmul(
                                    py, lhsT=onesk,
                                    rhs=b2r[0:1, dd * 512:(dd + 1) * 512],
                                    start=False, stop=True,
                                    skip_group_check=True)
                        for k, (ch, py) in enumerate(zip(grp, pys)):
                            sub = ch * (SC // 128) + s3
                            dst = yos[ch][:, dd * 512:(dd + 1) * 512]
                            if (sub * DD + dd) % 2 == 0:
                                nc.scalar.activation(dst, py, AF.Copy,
                                                     scale=wgc[:, sub:sub + 1])
                            else:
                                nc.vector.tensor_scalar(dst, py,
                                                        wgc[:, sub:sub + 1],
                                                        None, op0=ALU.mult)
                    for ch in grp:
                        sub = ch * (SC // 128) + s3
                        nc.sync.dma_start(
                            y_d[sub * 128:(sub + 1) * 128, :], yos[ch])
            sB.close()

    nc.compile()
    return nc


# --------------------------------------------------------------------------
# Host-side prep / orchestration
# --------------------------------------------------------------------------
_BUILT = {}


def _get(key, builder):
    if key not in _BUILT:
        _BUILT[key] = builder()
    return _BUILT[key]


def prep_l1(x, u1, gate_w, gate_b):
    x = np.asarray(x, np.float32)
    xb = x.astype(bf16)
    xlo = (x - xb.astype(np.float32)).astype(bf16)
    u1c = (np.asarray(u1, np.float32).transpose(1, 0, 2).reshape(D, ER)
           .astype(bf16))
    u1L = np.ascontiguousarray(
        u1c.reshape(DC, 128, ER).transpose(1, 0, 2)).reshape(128, DC * ER)
    gwf = np.asarray(gate_w, np.float32).T          # [D, E]
    gwhi = gwf.astype(bf16)
    gwlo = (gwf - gwhi.astype(np.float32)).astype(bf16)
    gwcat = np.concatenate(
        [gwhi.reshape(DC, 128, E), gwlo.reshape(DC, 128, E)], axis=2)
    gwL = np.ascontiguousarray(
        gwcat.transpose(1, 0, 2)).reshape(128, DC * 2 * E)
    gbb = np.ascontiguousarray(np.broadcast_to(
        np.asarray(gate_b, np.float32).reshape(1, E), (128, E)))
    maps = []
    for c in range(NCORES):
        def lay(a):
            s = np.ascontiguousarray(a[c * NT:(c + 1) * NT].T)  # [D, NT]
            return np.ascontiguousarray(
                s.reshape(DC, 128, NT).transpose(1, 0, 2)).reshape(128, DC * NT)
        maps.append(dict(xt=lay(xb), xtl=lay(xlo), u1c=u1L, gwt=gwL, gbb=gbb))
    return maps


def route(w_full):
    """Token lists per expert from the device-computed weights."""
    idxs, wgs = [], []
    for e in range(E):
        idx = np.nonzero(w_full[:, e] > 0)[0]
        idxs.append(idx)
        wgs.append(w_full[idx, e])
    maxc = max(len(i) for i in idxs)
    nch = max(2, -(-maxc // SC))
    return idxs, wgs, nch


def prep_l2(t1_full, idxs, wgs, nch, v1, b1, u2, v2, b2):
    npair = (nch + 1) // 2
    cap = nch * SC
    nsub = nch * (SC // 128)
    v1 = np.asarray(v1, np.float32)
    u2 = np.asarray(u2, np.float32)
    v2 = np.asarray(v2, np.float32)
    b1 = np.asarray(b1, np.float32)
    b2 = np.asarray(b2, np.float32)
    onesk = np.ones((1, 128), dtype=bf16)
    maps = []
    for e in range(E):
        idx, wg = idxs[e], wgs[e]
        pad = np.zeros((cap, R), dtype=bf16)
        pad[:len(idx)] = t1_full[idx, e * R:(e + 1) * R]
        arr = pad.reshape(nch, SC, R).transpose(0, 2, 1)  # [nch, R, SC]
        t1g = np.zeros((128, npair, SC), dtype=bf16)
        for ch in range(nch):
            t1g[(ch % 2) * 64:(ch % 2) * 64 + 64, ch // 2, :] = arr[ch]
        wp = np.zeros((cap,), np.float32)
        wp[:len(idx)] = wg
        maps.append(dict(
            t1g=t1g.reshape(128, npair * SC),
            v1d=np.concatenate([v1[e], v1[e]], 0).astype(bf16),
            u2l=np.ascontiguousarray(
                u2[e].reshape(HC, 128, R).transpose(1, 0, 2)
            ).reshape(128, HC * R).astype(bf16),
            v2d=np.concatenate([v2[e], v2[e]], 0).astype(bf16),
            b1l=np.ascontiguousarray(b1[e].reshape(HC, 128).T),
            b2r=b2[e].reshape(1, D).astype(bf16),
            wgc=np.ascontiguousarray(wp.reshape(nsub, 128).T),
            onesk=onesk,
        ))
    return maps


def run(inputs, return_info=False):
    import concourse.bass_utils as bass_utils

    x = np.asarray(inputs["x"], np.float32)
    l1_maps = prep_l1(x, inputs["u1"], inputs["gate_w"], inputs["gate_b"])
    nc1 = _get(("l1", 0), lambda: build_l1(0))
    res1 = bass_utils.run_bass_kernel_spmd(
        nc1, l1_maps, core_ids=list(range(NCORES)))
    t1_full = np.concatenate([r["t1"] for r in res1.results], axis=0)
    w_full = np.concatenate(
        [r["w"] for r in res1.results], axis=0).astype(np.float32)

    idxs, wgs, nch = route(w_full)
    l2_maps = prep_l2(t1_full, idxs, wgs, nch,
                      inputs["v1"], inputs["b1"], inputs["u2"],
                      inputs["v2"], inputs["b2"])
    with_b2 = not np.all(np.asarray(inputs["b2"]) == 0)
    nc2 = _get(("l2", nch, 0, with_b2), lambda: build_l2(nch, 0, with_b2))
    res2 = bass_utils.run_bass_kernel_spmd(
        nc2, l2_maps, core_ids=list(range(NCORES)))

    out = np.zeros((N_TOK, D), np.float32)
    for e in range(E):
        ye = np.asarray(res2.results[e]["y"])[:len(idxs[e])]
        out[idxs[e]] += ye.astype(np.float32)
    if return_info:
        return out, dict(l1_maps=l1_maps, l2_maps=l2_maps, nch=nch,
                         with_b2=with_b2)
    return out


def kernel(**inputs) -> np.ndarray:
    return run(inputs)


if __name__ == "__main__":
    nc1 = build_l1(0)
    nc2 = build_l2(3, 0)
    print("built ok")


# revision 28
# speedup vs baseline: 1.0482x; 1.0482x over previous
"""Trainium2 Bass kernel for nn_MixtureOfRanksLayer (moe_routing).

Two-launch expert-parallel design (device does ALL the math; the host only
moves data between launches — slicing, gathering rows by the device-computed
routing, and summing per-expert partial outputs, i.e. the unshard step):

LAUNCH 1 — data-parallel over tokens (512/core on 8 cores):
  T1  = x @ u1           [512, E*R=512]   rank projections for all experts
  lg  = x @ gate_w.T+b   [512, 8]         gate logits (same lhsT tiles)
  w   = top2-renormalized weights (masked-max + sigmoid(l1-l2), exact,
        identical math to the softmax-top2-renorm reference)
  outputs: T1 (bf16), w (f32, nonzero exactly at the top-2 experts)

HOST between launches: from w>0 derive each expert's token list; gather the
64 T1 columns of expert e at its tokens into a [cap, 64] block (cap =
per-expert capacity rounded to 384-slot chunks, computed from the actual
counts so it never overflows); lay out chunk c on SBUF partition half c%2
so the rank-64 matmuls of consecutive chunks run CONCURRENTLY on the two
row/column halves of the PE array (measured: tiled pairs stream at full
aggregate rate, one 384-row pair costs one stream).

LAUNCH 2 — expert-parallel (core e owns expert e, weights are NOT
replicated: 3.9MB/core instead of 21MB, the DMA roofline win):
  h   = relu(v1.T @ T1g + b1)   per hc chunk, evac ACT/DVE alternating
  T2  = u2.T @ h                 accumulated over hc (col-tiled halves)
  y   = (T2.T @ v2 + b2) * w     w applied as per-partition scale at evac
  output: y [cap, 2048] bf16

HOST: out[idx_e] += y_e[:count]  (the expert-parallel unshard/combine).

Measured HW facts this design is built on (microbench, 8xNC-v3):
  bf16 matmul ~0.345ns/row; fp32r is 2.8x SLOWER (so everything is bf16);
  row/col-tiled K=64/M=64 pairs at different tile positions stream
  concurrently; per-core DMA ~180B/ns; ACT/DVE evac ~1 col/cycle (PSUM
  source blocks DVE 2x modes) which makes h-evacuation the main non-PE
  cost: minimized by capacity (~1152 slots vs 4096 dense tokens/expert).
"""

from contextlib import ExitStack, nullcontext

import ml_dtypes
import numpy as np

import concourse.bacc as bacc
import concourse.mybir as mybir
import concourse.tile as tile

dt = mybir.dt
AF = mybir.ActivationFunctionType
ALU = mybir.AluOpType
AX = mybir.AxisListType

E, D, H, R = 8, 2048, 8192, 64
N_TOK = 4096
NCORES = 8
NT = N_TOK // NCORES     # launch-1 tokens per core
DC = D // 128            # contraction chunks over d_model
ER = E * R               # stacked expert-rank axis
HC = H // 128            # hidden chunks
SC = 384                 # launch-2 slot chunk (psum: 384 f32 = 1.5KB/bank)
TC = NT // 128           # launch-1 token chunks

bf16 = ml_dtypes.bfloat16


# --------------------------------------------------------------------------
# Launch 1: T1 + routing weights, data-parallel
# --------------------------------------------------------------------------
def build_l1(rep=0):
    f32 = dt.float32
    bf = dt.bfloat16
    nc = bacc.Bacc("TRN2", debug=False)

    xt_d = nc.dram_tensor("xt", [128, DC * NT], bf, kind="ExternalInput").ap()
    xl_d = nc.dram_tensor("xtl", [128, DC * NT], bf, kind="ExternalInput").ap()
    u1_d = nc.dram_tensor("u1c", [128, DC * ER], bf, kind="ExternalInput").ap()
    gw_d = nc.dram_tensor("gwt", [128, DC * 2 * E], bf, kind="ExternalInput").ap()
    gb_d = nc.dram_tensor("gbb", [128, E], f32, kind="ExternalInput").ap()
    t1_d = nc.dram_tensor("t1", [NT, ER], bf, kind="ExternalOutput").ap()
    w_d = nc.dram_tensor("w", [NT, E], f32, kind="ExternalOutput").ap()

    with ExitStack() as ctx:
        tc = ctx.enter_context(tile.TileContext(nc))
        const = ctx.enter_context(tc.tile_pool(name="const", bufs=1))
        sm = ctx.enter_context(tc.tile_pool(name="sm", bufs=2))
        outp = ctx.enter_context(tc.tile_pool(name="outp", bufs=2))
        ps_t1 = ctx.enter_context(tc.tile_pool(name="ps_t1", bufs=2, space="PSUM"))
        ps_lg = ctx.enter_context(tc.tile_pool(name="ps_lg", bufs=2, space="PSUM"))

        loop = tc.For_i(0, rep) if rep else nullcontext()
        with loop:
            xt = const.tile([128, DC, NT], bf, tag="xt")
            nc.sync.dma_start(xt, xt_d)
            xtl = const.tile([128, DC, NT], bf, tag="xtl")
            nc.sync.dma_start(xtl, xl_d)
            u1 = const.tile([128, DC, ER], bf, tag="u1")
            nc.sync.dma_start(u1, u1_d)
            gw = const.tile([128, DC, 2 * E], bf, tag="gw")
            nc.sync.dma_start(gw, gw_d)
            gbb = const.tile([128, E], f32, tag="gbb")
            nc.sync.dma_start(gbb, gb_d)

            for t in range(TC):
                pt = ps_t1.tile([128, ER], f32, tag="pt")
                pl = ps_lg.tile([128, 2 * E], f32, tag="pl")
                pl3 = ps_lg.tile([128, E], f32, tag="pl3")
                for c in range(DC):
                    lhs = xt[:, c, t * 128:(t + 1) * 128]
                    nc.tensor.matmul(pt, lhsT=lhs, rhs=u1[:, c, :],
                                     start=(c == 0), stop=(c == DC - 1),
                                     skip_group_check=True)
                    # logits in split-bf16: xhi@[gwhi|gwlo] and xlo@gwhi
                    nc.tensor.matmul(pl, lhsT=lhs, rhs=gw[:, c, :],
                                     start=(c == 0), stop=(c == DC - 1),
                                     skip_group_check=True)
                    nc.tensor.matmul(pl3,
                                     lhsT=xtl[:, c, t * 128:(t + 1) * 128],
                                     rhs=gw[:, c, 0:E],
                                     start=(c == 0), stop=(c == DC - 1),
                                     skip_group_check=True)
                t1o = outp.tile([128, ER], bf, tag="t1o")
                nc.vector.tensor_copy(t1o, pt)
                nc.sync.dma_start(t1_d[t * 128:(t + 1) * 128, :], t1o)

                # routing: exact top-2 renormalized softmax weights
                lgs = sm.tile([128, 2 * E], f32, tag="lgs")
                nc.vector.tensor_copy(lgs, pl)
                lg1 = sm.tile([128, E], f32, tag="lg1")
                nc.vector.tensor_tensor(lg1, pl3, gbb, op=ALU.add)
                lg0 = sm.tile([128, E], f32, tag="lg0")
                nc.vector.tensor_tensor(lg0, lgs[:, 0:E], lgs[:, E:2 * E],
                                        op=ALU.add)
                lg = sm.tile([128, E], f32, tag="lg")
                nc.vector.tensor_add(lg, lg0, lg1)
                l1 = sm.tile([128, 1], f32, tag="l1")
                nc.vector.reduce_max(out=l1, in_=lg, axis=AX.X)
                m1t = sm.tile([128, E], f32, tag="m1t")
                nc.vector.tensor_scalar(m1t, lg, l1, None, op0=ALU.is_equal)
                lm = sm.tile([128, E], f32, tag="lm")
                nc.vector.tensor_scalar(lm, m1t, -1e30, None, op0=ALU.mult)
                nc.vector.tensor_add(lm, lm, lg)
                l2 = sm.tile([128, 1], f32, tag="l2")
                nc.vector.reduce_max(out=l2, in_=lm, axis=AX.X)
                m2t = sm.tile([128, E], f32, tag="m2t")
                nc.vector.tensor_scalar(m2t, lm, l2, None, op0=ALU.is_equal)
                dif = sm.tile([128, 1], f32, tag="dif")
                nc.vector.tensor_sub(dif, l1, l2)
                s1v = sm.tile([128, 1], f32, tag="s1v")
                nc.scalar.activation(s1v, dif, AF.Sigmoid)
                s0v = sm.tile([128, 1], f32, tag="s0v")
                nc.scalar.activation(s0v, dif, AF.Sigmoid, scale=-1.0)
                wa = sm.tile([128, E], f32, tag="wa")
                nc.vector.tensor_scalar(wa, m1t, s1v, None, op0=ALU.mult)
                wb_ = sm.tile([128, E], f32, tag="wb_")
                nc.vector.tensor_scalar(wb_, m2t, s0v, None, op0=ALU.mult)
                wt = outp.tile([128, E], f32, tag="wt")
                nc.vector.tensor_add(wt, wa, wb_)
                nc.sync.dma_start(w_d[t * 128:(t + 1) * 128, :], wt)

    nc.compile()
    return nc


# --------------------------------------------------------------------------
# Launch 2: per-expert FFN over gathered slots, expert-parallel
# --------------------------------------------------------------------------
def build_l2(nch, rep=0, with_b2=True, hoist=False):
    f32 = dt.float32
    bf = dt.bfloat16
    npair = (nch + 1) // 2
    cap = nch * SC
    nsub = nch * (SC // 128)
    DD = D // 512
    nc = bacc.Bacc("TRN2", debug=False)

    t1g_d = nc.dram_tensor("t1g", [128, npair * SC], bf, kind="ExternalInput").ap()
    v1_d = nc.dram_tensor("v1d", [128, H], bf, kind="ExternalInput").ap()
    u2_d = nc.dram_tensor("u2l", [128, HC * R], bf, kind="ExternalInput").ap()
    v2_d = nc.dram_tensor("v2d", [128, D], bf, kind="ExternalInput").ap()
    b1_d = nc.dram_tensor("b1l", [128, HC], f32, kind="ExternalInput").ap()
    b2_d = nc.dram_tensor("b2r", [1, D], bf, kind="ExternalInput").ap()
    wg_d = nc.dram_tensor("wgc", [128, nsub], f32, kind="ExternalInput").ap()
    on_d = nc.dram_tensor("onesk", [1, 128], bf, kind="ExternalInput").ap()
    y_d = nc.dram_tensor("y", [cap, D], bf, kind="ExternalOutput").ap()

    # chunk -> (pair tile index, partition half)
    halves = [(ch // 2, (ch % 2) * 64) for ch in range(nch)]

    with ExitStack() as ctx:
        tc = ctx.enter_context(tile.TileContext(nc))
        const = ctx.enter_context(tc.tile_pool(name="const", bufs=1))
        t2p = ctx.enter_context(tc.tile_pool(name="t2p", bufs=1))
        hsb = ctx.enter_context(tc.tile_pool(name="hsb", bufs=6))
        ysb = ctx.enter_context(tc.tile_pool(name="ysb", bufs=4))

        def load_consts():
            t1g = const.tile([128, npair, SC], bf, tag="t1g")
            nc.sync.dma_start(t1g, t1g_d)
            v1d = const.tile([128, H], bf, tag="v1d")
            nc.sync.dma_start(v1d, v1_d)
            u2l = const.tile([128, HC, R], bf, tag="u2l")
            nc.sync.dma_start(u2l, u2_d)
            u2l2 = const.tile([128, HC, R], bf, tag="u2l2")
            nc.sync.dma_start(u2l2, u2_d)
            v2d = const.tile([128, D], bf, tag="v2d")
            nc.sync.dma_start(v2d, v2_d)
            b1l = const.tile([128, HC], f32, tag="b1l")
            nc.sync.dma_start(b1l, b1_d)
            b2r = const.tile([1, D], bf, tag="b2r")
            nc.sync.dma_start(b2r, b2_d)
            wgc = const.tile([128, nsub], f32, tag="wgc")
            nc.sync.dma_start(wgc, wg_d)
            onesk = const.tile([1, 128], bf, tag="onesk")
            nc.sync.dma_start(onesk, on_d)
            return t1g, v1d, u2l, u2l2, v2d, b1l, b2r, wgc, onesk

        if hoist:
            t1g, v1d, u2l, u2l2, v2d, b1l, b2r, wgc, onesk = load_consts()
        loop = tc.For_i(0, rep) if rep else nullcontext()
        with loop:
            if not hoist:
                t1g, v1d, u2l, u2l2, v2d, b1l, b2r, wgc, onesk = load_consts()

            t2sb = t2p.tile([128, npair, SC], bf, tag="t2sb")

            # ---- phase A: h = relu(v1.T @ t1g + b1); T2 += u2.T @ h ----
            with ExitStack() as sA:
                ps_ha = sA.enter_context(
                    tc.tile_pool(name="ps_ha", bufs=3, space="PSUM"))
                ps_hb = sA.enter_context(
                    tc.tile_pool(name="ps_hb", bufs=3, space="PSUM"))
                ps_t2 = sA.enter_context(
                    tc.tile_pool(name="ps_t2", bufs=1, space="PSUM"))

                # software pipeline: m2+evac for hc runs DEPTH chunks ahead
                # of m3(hc), so the in-order PE never blocks on an evac it
                # just scheduled (evac ~525ns+2 sems vs ~250ns of PE work/hc)
                DEPTH = 3
                for pj in range(npair):
                    has_b = 2 * pj + 1 < nch
                    # separate PSUM banks per column half (same-bank col-tiled
                    # accumulation serializes the PE; separate banks stream)
                    pt2a = ps_t2.tile([128, SC], f32, tag="pt2a", name="pt2a")
                    pt2b = None
                    if has_b:
                        pt2b = ps_t2.tile([128, SC], f32, tag="pt2b", name="pt2b")
                    hq = {}
                    for step in range(HC + DEPTH):
                        if step < HC:
                            hc = step
                            hs = slice(hc * 128, (hc + 1) * 128)
                            pha = ps_ha.tile([128, SC], f32, tag="ha", name="ha")
                            nc.tensor.matmul(pha, lhsT=v1d[0:64, hs],
                                             rhs=t1g[0:64, pj, :],
                                             start=True, stop=True)
                            if has_b:
                                phb = ps_hb.tile([128, SC], f32, tag="hb",
                                                 name="hb")
                                nc.tensor.matmul(phb, lhsT=v1d[64:128, hs],
                                                 rhs=t1g[64:128, pj, :],
                                                 start=True, stop=True)
                            ha = hsb.tile([128, SC], bf, tag="ha", name="sha")
                            bias = b1l[:, hc:hc + 1]
                            if hc % 2 == 0:
                                nc.scalar.activation(ha, pha, AF.Relu, bias=bias)
                            else:
                                nc.vector.tensor_scalar(ha, pha, bias, 0.0,
                                                        op0=ALU.add, op1=ALU.max)
                            hb = None
                            if has_b:
                                hb = hsb.tile([128, SC], bf, tag="hb", name="shb")
                                if hc % 2 == 0:
                                    nc.vector.tensor_scalar(hb, phb, bias, 0.0,
                                                            op0=ALU.add,
                                                            op1=ALU.max)
                                else:
                                    nc.scalar.activation(hb, phb, AF.Relu,
                                                         bias=bias)
                            hq[hc] = (ha, hb)
                        mc = step - DEPTH
                        if mc >= 0:
                            ha, hb = hq.pop(mc)
                            nc.tensor.matmul(pt2a[0:64, :], lhsT=u2l[:, mc, :],
                                             rhs=ha,
                                             start=(mc == 0), stop=(mc == HC - 1),
                                             skip_group_check=True)
                            if hb is not None:
                                nc.tensor.matmul(pt2b[64:128, :],
                                                 lhsT=u2l2[:, mc, :], rhs=hb,
                                                 start=(mc == 0),
                                                 stop=(mc == HC - 1),
                                                 skip_group_check=True)
                    nc.vector.tensor_copy(t2sb[0:64, pj, :], pt2a[0:64, :])
                    if has_b:
                        nc.vector.tensor_copy(t2sb[64:128, pj, :],
                                              pt2b[64:128, :])

            # ---- phase B: y = (T2.T @ v2 + b2) * w ----
            # b2 matmuls only when b2 != 0 (K=1 LDWEIGHTS thrash is pricey);
            # one wide [128, D] output tile per slot-sub so the SP queue
            # dispatches 1 DMA/sub instead of 4 (565ns dispatch each)
            sB = ExitStack()
            ps_y = sB.enter_context(
                tc.tile_pool(name="ps_y", bufs=4, space="PSUM"))
            groups = [tuple(c for c in (2 * i, 2 * i + 1) if c < nch)
                      for i in range((nch + 1) // 2)]
            for grp in groups:
                for s3 in range(SC // 128):
                    yos = {}
                    for ch in grp:
                        sub = ch * (SC // 128) + s3
                        yos[ch] = ysb.tile([128, D], bf, tag="yo",
                                           name=f"yo{sub}")
                    for dd in range(DD):
                        pys = []
                        for ch in grp:
                            pj, half = halves[ch]
                            py = ps_y.tile([128, 512], f32, tag="py", name="py")
                            nc.tensor.matmul(
                                py,
                                lhsT=t2sb[half:half + 64, pj,
                                          s3 * 128:(s3 + 1) * 128],
                                rhs=v2d[half:half + 64, dd * 512:(dd + 1) * 512],
                                start=True, stop=not with_b2,
                                skip_group_check=True)
                            pys.append(py)
                        if with_b2:
                            for ch, py in zip(grp, pys):
                                nc.tensor.matmul(
                                    py, lhsT=onesk,
                                    rhs=b2r[0:1, dd * 512:(dd + 1) * 512],
                                    start=False, stop=True,
                                    skip_group_check=True)
                        for k, (ch, py) in enumerate(zip(grp, pys)):
                            sub = ch * (SC // 128) + s3
                            dst = yos[ch][:, dd * 512:(dd + 1) * 512]
                            if (sub * DD + dd) % 2 == 0:
                                nc.scalar.activation(dst, py, AF.Copy,
                                                     scale=wgc[:, sub:sub + 1])
                            else:
                                nc.vector.tensor_scalar(dst, py,
                                                        wgc[:, sub:sub + 1],
                                                        None, op0=ALU.mult)
                    for ch in grp:
                        sub = ch * (SC // 128) + s3
                        nc.sync.dma_start(
                            y_d[sub * 128:(sub + 1) * 128, :], yos[ch])
            sB.close()

    nc.compile()
    return nc


# --------------------------------------------------------------------------
# Host-side prep / orchestration
# --------------------------------------------------------------------------
_BUILT = {}


def _get(key, builder):
    if key not in _BUILT:
        _BUILT[key] = builder()
    return _BUILT[key]


def prep_l1(x, u1, gate_w, gate_b):
    x = np.asarray(x, np.float32)
    xb = x.astype(bf16)
    xlo = (x - xb.astype(np.float32)).astype(bf16)
    u1c = (np.asarray(u1, np.float32).transpose(1, 0, 2).reshape(D, ER)
           .astype(bf16))
    u1L = np.ascontiguousarray(
        u1c.reshape(DC, 128, ER).transpose(1, 0, 2)).reshape(128, DC * ER)
    gwf = np.asarray(gate_w, np.float32).T          # [D, E]
    gwhi = gwf.astype(bf16)
    gwlo = (gwf - gwhi.astype(np.float32)).astype(bf16)
    gwcat = np.concatenate(
        [gwhi.reshape(DC, 128, E), gwlo.reshape(DC, 128, E)], axis=2)
    gwL = np.ascontiguousarray(
        gwcat.transpose(1, 0, 2)).reshape(128, DC * 2 * E)
    gbb = np.ascontiguousarray(np.broadcast_to(
        np.asarray(gate_b, np.float32).reshape(1, E), (128, E)))
    maps = []
    for c in range(NCORES):
        def lay(a):
            s = np.ascontiguousarray(a[c * NT:(c + 1) * NT].T)  # [D, NT]
            return np.ascontiguousarray(
                s.reshape(DC, 128, NT).transpose(1, 0, 2)).reshape(128, DC * NT)
        maps.append(dict(xt=lay(xb), xtl=lay(xlo), u1c=u1L, gwt=gwL, gbb=gbb))
    return maps


def route(w_full):
    """Token lists per expert from the device-computed weights."""
    idxs, wgs = [], []
    for e in range(E):
        idx = np.nonzero(w_full[:, e] > 0)[0]
        idxs.append(idx)
        wgs.append(w_full[idx, e])
    maxc = max(len(i) for i in idxs)
    nch = max(2, -(-maxc // SC))
    return idxs, wgs, nch


def prep_l2(t1_full, idxs, wgs, nch, v1, b1, u2, v2, b2):
    npair = (nch + 1) // 2
    cap = nch * SC
    nsub = nch * (SC // 128)
    v1 = np.asarray(v1, np.float32)
    u2 = np.asarray(u2, np.float32)
    v2 = np.asarray(v2, np.float32)
    b1 = np.asarray(b1, np.float32)
    b2 = np.asarray(b2, np.float32)
    onesk = np.ones((1, 128), dtype=bf16)
    maps = []
    for e in range(E):
        idx, wg = idxs[e], wgs[e]
        pad = np.zeros((cap, R), dtype=bf16)
        pad[:len(idx)] = t1_full[idx, e * R:(e + 1) * R]
        arr = pad.reshape(nch, SC, R).transpose(0, 2, 1)  # [nch, R, SC]
        t1g = np.zeros((128, npair, SC), dtype=bf16)
        for ch in range(nch):
            t1g[(ch % 2) * 64:(ch % 2) * 64 + 64, ch // 2, :] = arr[ch]
        wp = np.zeros((cap,), np.float32)
        wp[:len(idx)] = wg
        maps.append(dict(
            t1g=t1g.reshape(128, npair * SC),
            v1d=np.concatenate([v1[e], v1[e]], 0).astype(bf16),
            u2l=np.ascontiguousarray(
                u2[e].reshape(HC, 128, R).transpose(1, 0, 2)
            ).reshape(128, HC * R).astype(bf16),
            v2d=np.concatenate([v2[e], v2[e]], 0).astype(bf16),
            b1l=np.ascontiguousarray(b1[e].reshape(HC, 128).T),
            b2r=b2[e].reshape(1, D).astype(bf16),
            wgc=np.ascontiguousarray(wp.reshape(nsub, 128).T),
            onesk=onesk,
        ))
    return maps


def run(inputs, return_info=False):
    import concourse.bass_utils as bass_utils

    x = np.asarray(inputs["x"], np.float32)
    l1_maps = prep_l1(x, inputs["u1"], inputs["gate_w"], inputs["gate_b"])
    nc1 = _get(("l1", 0), lambda: build_l1(0))
    res1 = bass_utils.run_bass_kernel_spmd(
        nc1, l1_maps, core_ids=list(range(NCORES)))
    t1_full = np.concatenate([r["t1"] for r in res1.results], axis=0)
    w_full = np.concatenate(
        [r["w"] for r in res1.results], axis=0).astype(np.float32)

    idxs, wgs, nch = route(w_full)
    l2_maps = prep_l2(t1_full, idxs, wgs, nch,
                      inputs["v1"], inputs["b1"], inputs["u2"],
                      inputs["v2"], inputs["b2"])
    with_b2 = not np.all(np.asarray(inputs["b2"]) == 0)
    nc2 = _get(("l2", nch, 0, with_b2), lambda: build_l2(nch, 0, with_b2))
    res2 = bass_utils.run_bass_kernel_spmd(
        nc2, l2_maps, core_ids=list(range(NCORES)))

    out = np.zeros((N_TOK, D), np.float32)
    for e in range(E):
        ye = np.asarray(res2.results[e]["y"])[:len(idxs[e])]
        out[idxs[e]] += ye.astype(np.float32)
    if return_info:
        return out, dict(l1_maps=l1_maps, l2_maps=l2_maps, nch=nch,
                         with_b2=with_b2)
    return out


def kernel(**inputs) -> np.ndarray:
    return run(inputs)


if __name__ == "__main__":
    nc1 = build_l1(0)
    nc2 = build_l2(3, 0)
    print("built ok")
